# revision 2
# baseline (speedup 1.0000x reference)
"""Trainium2 Bass kernel for causal self-attention with RoPE.

Problem: x[2,2048,2048] f32, w_qkv[6144,2048], w_out[2048,2048].
  qkv = x @ w_qkv.T ; split into 16 heads of 128; RoPE on q,k;
  causal softmax attention; out = attn_out @ w_out.T.

Sharding (8 cores): core c -> batch b = c//4, head-group g = c%4
(4 heads each). Each core computes a partial output projection for its
heads; the host sums the 4 partials per batch.

Per-core schedule (engineered against the TimelineSim cost model):

Phase A (fused projections, one streamed pass over x^T in bf16):
  per 256-token group: 8 q/k psum tiles (fp32 PSUM) with RoPE fused at
  eviction -- rotate_half is a 64-partition window swap on DVE (sign
  folded into the host sin table), rotated q/k stored bf16 -- plus 2 v
  psum tiles reusing the same x^T SBUF tiles (v stored bf16 in natural
  [t, hd] layout with a ones column at index 128 for the softmax
  denominator). The v groups and x^T prefetches are software-pipelined
  two token-groups behind the q/k stream (the last two v groups weave
  into phase C's warmup ticks); weight/x loads are ordered so the cold
  start is transfer-bound only.

Phase C (attention + output projection, one flat pipelined stream):
  for each (q-group G, head h): scoresT[tk,tq] = krot.T @ qrot in bf16
  (diagonal tiles width-trimmed), Exp on ACT -> bf16 with causal zeroing
  of the diagonal tile via Pool affine_select, then AV matmuls (129 wide,
  bf16) accumulate [tq, 0:129] in PSUM; col 128 is the denominator.
  Emission is tick-pipelined: AV lags scores by 3 ticks (hides the exp
  chain), per-column-tile eviction (reciprocal + scale on DVE, bf16 PE
  transpose) lags its AV stop by 4 ticks, and the previous q-group's
  output projection (bf16 weights, fp32 PSUM) weaves into every other
  tick. The stream's last head switches to narrow per-gi columns for its
  diagonal region so its evictions + projections interleave instead of
  serializing in the drain.

fp32r/bf16 both run 1 PE cycle/row (moving operand >= 256 wide for
fp32r; any width for bf16). All matmuls accumulate in fp32 PSUM; the
attention core rounds q,k,p,v,y to bf16 (measured rel l2 vs fp64
reference: 5.9e-3).
"""

import os
import sys
import time
from contextlib import ExitStack

import ml_dtypes
import numpy as np

if "/opt/trn_rl_repo" not in sys.path:
    sys.path.insert(0, "/opt/trn_rl_repo")

import concourse.bass as bass  # noqa: E402
import concourse.mybir as mybir  # noqa: E402
import concourse.tile as tile  # noqa: E402
from concourse import bacc  # noqa: E402
from concourse import bass_utils  # noqa: E402
from concourse.masks import make_identity  # noqa: E402

P = 128
T = 2048
DIM = 2048
HD = 128
NH = 4  # heads per core
TGQ = 256  # t-group width for phase A (= fp32r full-rate minimum)
QG = 512  # tq group width in attention
WSCALE = 32.0  # host-side fp8 scale on w_qkv rows
SCALE = float(HD) ** -0.5 / (WSCALE * WSCALE)
F32 = mybir.dt.float32
F32R = mybir.dt.float32r
BF16 = mybir.dt.bfloat16
FP8 = mybir.dt.float8e4
DR = mybir.MatmulPerfMode.DoubleRow
NDP = 8  # DoubleRow d-pairs
VBW = HD + 1  # v block width per (t-tile, head): 128 data + 1 ones col

_CACHE: dict = {}
LAST_RESULTS = None


def _build_program(t=T, dim=DIM):
    ndt = dim // P  # contraction tiles over D (16)
    ntt = t // P  # token tiles (16)
    ntg = t // TGQ  # 8
    nqg = t // QG  # 4
    nog = dim // 512  # 4

    nc = bacc.Bacc("TRN2", target_bir_lowering=False, debug=False)

    xth_d = nc.dram_tensor("xth", [dim, t], FP8, kind="ExternalInput")
    xtl_d = nc.dram_tensor("xtl", [dim, t], FP8, kind="ExternalInput")
    wqh_d = nc.dram_tensor("wqh", [dim, 2 * NH * HD], FP8, kind="ExternalInput")
    wql_d = nc.dram_tensor("wql", [dim, 2 * NH * HD], FP8, kind="ExternalInput")
    wvh_d = nc.dram_tensor("wvh", [dim, NH * HD], FP8, kind="ExternalInput")
    wvl_d = nc.dram_tensor("wvl", [dim, NH * HD], FP8, kind="ExternalInput")
    wo_d = nc.dram_tensor("wot", [NH * HD, dim], BF16, kind="ExternalInput")
    cos_d = nc.dram_tensor("cost", [P, t], F32, kind="ExternalInput")
    sin_d = nc.dram_tensor("sint", [P, t], F32, kind="ExternalInput")
    out_d = nc.dram_tensor("out", [t, dim], F32, kind="ExternalOutput")

    with ExitStack() as ctx:
        tc = ctx.enter_context(tile.TileContext(nc))

        const = ctx.enter_context(tc.tile_pool(name="const", bufs=1))
        ident = const.tile([P, P], BF16)

        # persistent rotated q/k (bf16) and v (bf16, natural layout)
        qk_pool = ctx.enter_context(tc.tile_pool(name="qk", bufs=1))
        qk_sb = qk_pool.tile([P, 2 * NH, t], BF16)
        v_pool = ctx.enter_context(tc.tile_pool(name="vb", bufs=1))
        v_sb = v_pool.tile([P, ntt * NH * VBW], BF16)

        # xta/wv outlive phase A: the last two v groups weave into the
        # attention stream's warmup ticks
        wv_pool = ctx.enter_context(tc.tile_pool(name="wv", bufs=1))
        xt_pool = ctx.enter_context(tc.tile_pool(name="xta", bufs=3))

        # ------- Phase A: fused q/k projection + RoPE + v projection -------
        with (
            tc.tile_pool(name="wqk", bufs=1) as wqk_pool,
            tc.tile_pool(name="trig", bufs=3) as trig_pool,
            tc.tile_pool(name="ropes", bufs=2) as rope_pool,
            tc.tile_pool(name="psqk", bufs=6, space="PSUM") as ps_qk_pool,
            tc.tile_pool(name="psv", bufs=2, space="PSUM") as ps_v_pool,
        ):
            wqh_sb = wqk_pool.tile([P, ndt, 2 * NH * HD], FP8)
            wql_sb = wqk_pool.tile([P, ndt, 2 * NH * HD], FP8)
            wvh_sb = wv_pool.tile([P, ndt, NH * HD], FP8)
            wvl_sb = wv_pool.tile([P, ndt, NH * HD], FP8)
            xth_sb = [
                xt_pool.tile([P, ndt * TGQ], FP8, name=f"xth_{k}", tag="xth")
                for k in range(ntg)
            ]
            xtl_sb = [
                xt_pool.tile([P, ndt * TGQ], FP8, name=f"xtl_{k}", tag="xtl")
                for k in range(ntg)
            ]

            def load_xt(k, halves=1, engines=None):
                t0 = k * TGQ
                step = ndt // halves
                for sb, dram in ((xth_sb, xth_d), (xtl_sb, xtl_d)):
                    dst = sb[k][:].rearrange("p (d c) -> p d c", c=TGQ)
                    for hh in range(halves):
                        d0 = hh * step
                        eng = nc.gpsimd if engines is None else engines[hh]
                        eng.dma_start(
                            dst[:, d0 : d0 + step, :],
                            dram.ap()[d0 * P : (d0 + step) * P, t0 : t0 + TGQ]
                            .rearrange("(d p) c -> p d c", p=P),
                        )

            def load_trig(k, eng=None):
                eng = eng or nc.gpsimd
                t0 = k * TGQ
                cos_t = trig_pool.tile([P, TGQ], F32, name=f"cos_{k}", tag="cos")
                sin_t = trig_pool.tile([P, TGQ], F32, name=f"sin_{k}", tag="sin")
                eng.dma_start(cos_t[:], cos_d.ap()[:, t0 : t0 + TGQ])
                eng.dma_start(sin_t[:], sin_d.ap()[:, t0 : t0 + TGQ])
                return cos_t, sin_t

            # constants first: every later v_sb/ident user depends on
            # them (tile-granular), keep them off the DMA-clogged queue
            nc.gpsimd.memset(
                v_sb[:].rearrange("p (a c) -> p a c", c=VBW)[:, :, HD : HD + 1],
                WSCALE,
            )
            make_identity(nc, ident[:])

            # cold-start DMA order: wqk col-wave 0, xt(tg0), waves 1-3,
            # xt(tg1), wv, xt(tg2); the rest stream during the tg loop.
            NW = 4
            wcol = 2 * NH * HD // NW
            wh_src = wqh_d.ap().rearrange("(d p) c -> p d c", p=P)
            wl_src = wql_d.ap().rearrange("(d p) c -> p d c", p=P)
            for w in range(NW):
                c0 = w * wcol
                nc.sync.dma_start(
                    wqh_sb[:, :, c0 : c0 + wcol], wh_src[:, :, c0 : c0 + wcol]
                )
                if w == 0:
                    load_xt(0, halves=2, engines=[nc.scalar, nc.sync])
                nc.scalar.dma_start(
                    wql_sb[:, :, c0 : c0 + wcol], wl_src[:, :, c0 : c0 + wcol]
                )
                if w == 0:
                    load_xt(1, halves=2, engines=[nc.sync, nc.scalar])
                    trig0 = load_trig(0)
                    trig1 = load_trig(1)
            load_xt(2)
            nc.gpsimd.dma_start(
                wvh_sb[:],
                wvh_d.ap().rearrange("(d p) c -> p d c", p=P),
            )
            nc.gpsimd.dma_start(
                wvl_sb[:],
                wvl_d.ap().rearrange("(d p) c -> p d c", p=P),
            )
            trigs = {0: trig0, 1: trig1}

            pend_rope = []  # (jt, raw, ps2 deferral) emitted with 1-group lag

            def emit_qk_group(k, jt, cos_t, sin_t):
                t0 = k * TGQ
                xh3 = xth_sb[k][:].rearrange("p (d c) -> p d c", c=TGQ)
                xl3 = xtl_sb[k][:].rearrange("p (d c) -> p d c", c=TGQ)
                cols = slice(jt * P, (jt + 1) * P)
                ps = ps_qk_pool.tile([P, TGQ], F32)
                terms = [(wqh_sb, xh3), (wql_sb, xh3), (wqh_sb, xl3)]
                for ti, (wsb, xsb) in enumerate(terms):
                    for dp in range(NDP):
                        nc.tensor.matmul(
                            ps[:],
                            wsb[:, 2 * dp : 2 * dp + 2, cols],
                            xsb[:, 2 * dp : 2 * dp + 2, :],
                            start=(ti == 0 and dp == 0),
                            stop=(ti == 2 and dp == NDP - 1),
                            perf_mode=DR,
                        )
                raw = rope_pool.tile([P, TGQ], F32R, tag="raw")
                nc.scalar.copy(raw[:], ps[:])
                pend_rope.append((k, jt, raw, cos_t, sin_t))

            def finish_rope():
                # rotate_half as a 64-partition window swap (sign folded
                # into the host sin table), replacing the PE S-matmul
                k, jt, raw, cos_t, sin_t = pend_rope.pop(0)
                t0 = k * TGQ
                rf = raw[:].bitcast(F32)
                sh = rope_pool.tile([P, TGQ], F32, tag="sh")
                nc.vector.tensor_copy(sh[0 : P // 2, :], rf[P // 2 : P, :])
                nc.vector.tensor_copy(sh[P // 2 : P, :], rf[0 : P // 2, :])
                t1 = rope_pool.tile([P, TGQ], F32, tag="t1")
                nc.vector.tensor_mul(t1[:], sh[:], sin_t[:])
                t2 = rope_pool.tile([P, TGQ], F32, tag="t2")
                nc.gpsimd.tensor_mul(t2[:], raw[:].bitcast(F32), cos_t[:])
                nc.vector.tensor_add(qk_sb[:, jt, t0 : t0 + TGQ], t2[:], t1[:])

            def emit_v_group(k, sb):
                tt = k * 2 + sb
                xh3 = xth_sb[k][:].rearrange("p (d c) -> p d c", c=TGQ)
                xl3 = xtl_sb[k][:].rearrange("p (d c) -> p d c", c=TGQ)
                cols = slice(sb * P, (sb + 1) * P)
                ps = ps_v_pool.tile([P, NH * HD], F32, name=f"psv_{tt}", tag="psv")
                terms = [(xh3, wvh_sb), (xh3, wvl_sb), (xl3, wvh_sb)]
                for ti, (xsb, wsb) in enumerate(terms):
                    for dp in range(NDP):
                        nc.tensor.matmul(
                            ps[:],
                            xsb[:, 2 * dp : 2 * dp + 2, cols],
                            wsb[:, 2 * dp : 2 * dp + 2, :],
                            start=(ti == 0 and dp == 0),
                            stop=(ti == 2 and dp == NDP - 1),
                            perf_mode=DR,
                        )
                for h in range(NH):
                    off = (tt * NH + h) * VBW
                    nc.vector.tensor_copy(
                        v_sb[:, off : off + HD], ps[:, h * HD : (h + 1) * HD]
                    )

            # tg0/tg1 interleaved: each wqk column-wave feeds both token
            # groups, so the cold region is PE-bound instead of DMA-bound
            cos0, sin0 = trigs.pop(0)
            cos1, sin1 = trigs.pop(1)
            for jp in range(NH):
                if jp == 1:
                    trigs[2] = load_trig(2)
                if jp == 2:
                    trigs[3] = load_trig(3)
                for k in (0, 1):
                    for jt in (2 * jp, 2 * jp + 1):
                        emit_qk_group(
                            k, jt, cos0 if k == 0 else cos1,
                            sin0 if k == 0 else sin1,
                        )
                        if len(pend_rope) > 1:
                            finish_rope()
            for k in range(2, ntg):
                cos_t, sin_t = trigs.pop(k)
                for jt in range(2 * NH):
                    # v-projection weave: tg k>=2 carries v(k-2) at jts 1,3
                    if jt in (1, 3):
                        emit_v_group(k - 2, (jt - 1) // 2)
                        if jt == 3 and k + 1 < ntg:
                            load_xt(k + 1)
                    if jt == 2 and k + 2 < ntg:
                        trigs[k + 2] = load_trig(k + 2)
                    emit_qk_group(k, jt, cos_t, sin_t)
                    if len(pend_rope) > 1:
                        finish_rope()
            finish_rope()

        # normalized attn out Y^T per (head, q-group), bf16
        yt_pool = ctx.enter_context(tc.tile_pool(name="yt", bufs=1))
        yt_sb = [
            [
                yt_pool.tile([P, QG], BF16, name=f"yt_{h}_{G}", tag=f"yt{h}_{G}")
                for G in range(nqg)
            ]
            for h in range(NH)
        ]

        # ------------- Phase C: attention + output projection -------------
        with (
            tc.tile_pool(name="wo", bufs=1) as wo_pool,
            tc.tile_pool(name="expt", bufs=4) as exp_pool,
            tc.tile_pool(name="ynorm", bufs=6) as y_pool,
            tc.tile_pool(name="recip", bufs=6) as r_pool,
            tc.tile_pool(name="ob", bufs=8) as out_pool,
            tc.tile_pool(name="pss", bufs=2, space="PSUM") as ps_s_pool,
            tc.tile_pool(name="psy", bufs=4, space="PSUM") as ps_y_pool,
            tc.tile_pool(name="pso", bufs=2, space="PSUM") as ps_o_pool,
        ):
            wo_sb = wo_pool.tile([P, nog, NH, 512], BF16)
            for og in range(nog):
                nc.gpsimd.dma_start(
                    wo_sb[:, og, :, :],
                    wo_d.ap()[:, og * 512 : (og + 1) * 512].rearrange(
                        "(h p) c -> p h c", p=P
                    ),
                )

            # last two token-groups' v projection, chunked 4 matmuls per
            # tick across the warmup ticks (pso ring is idle until G1)
            vtail = [(ntg - 2, 0), (ntg - 2, 1), (ntg - 1, 0), (ntg - 1, 1)]
            vtail_ps = {}

            def emit_v_chunk(tick):
                gidx, c = divmod(tick, 4)
                k, sb = vtail[gidx]
                tt = k * 2 + sb
                xh3 = xth_sb[k][:].rearrange("p (d c) -> p d c", c=TGQ)
                xl3 = xtl_sb[k][:].rearrange("p (d c) -> p d c", c=TGQ)
                cols = slice(sb * P, (sb + 1) * P)
                if c == 0:
                    vtail_ps[gidx] = ps_o_pool.tile(
                        [P, NH * HD], F32, name=f"psvt_{tt}", tag="pso"
                    )
                ps = vtail_ps[gidx]
                terms = [(xh3, wvh_sb), (xh3, wvl_sb), (xl3, wvh_sb)]
                units = [(ti, dp) for ti in range(3) for dp in range(NDP)]
                for u in range(6 * c, 6 * c + 6):
                    ti, dp = units[u]
                    xsb, wsb = terms[ti]
                    nc.tensor.matmul(
                        ps[:],
                        xsb[:, 2 * dp : 2 * dp + 2, cols],
                        wsb[:, 2 * dp : 2 * dp + 2, :],
                        start=(u == 0),
                        stop=(u == 23),
                        perf_mode=DR,
                    )
                if c == 3:
                    for h in range(NH):
                        off = (tt * NH + h) * VBW
                        nc.vector.tensor_copy(
                            v_sb[:, off : off + HD], ps[:, h * HD : (h + 1) * HD]
                        )

            def emit_proj(Gp, tt, og):
                ps = ps_o_pool.tile([P, 512], F32, name=f"pso_{og}_{tt}", tag="pso")
                for h in range(NH):
                    nc.tensor.matmul(
                        ps[:],
                        yt_sb[h][Gp][:, (tt % 4) * P : (tt % 4 + 1) * P],
                        wo_sb[:, og, h, :],
                        start=(h == 0),
                        stop=(h == NH - 1),
                    )
                ob = out_pool.tile([P, 512], F32, tag="ob")
                nc.vector.tensor_copy(ob[:], ps[:])
                nc.sync.dma_start(
                    out_d.ap()[tt * P : (tt + 1) * P, og * 512 : (og + 1) * 512],
                    ob[:],
                )

            # --- flat software-pipelined stream over all (G, h, j) ---
            # last head: j>=4G ticks become narrow per-gi columns so its
            # evictions + projections interleave instead of draining late
            cols = []
            for G in range(nqg):
                for h in range(NH):
                    last = G == nqg - 1 and h == NH - 1
                    for j in range(4 * G if last else 4 * G + 4):
                        cols.append(("w", G, h, j, 0))
                    if last:
                        for gi in range(4):
                            for j in range(4 * G, 4 * G + gi + 1):
                                cols.append(("n", G, h, j, gi))
            N = len(cols)
            ps_y_t = {}  # (G,h) -> list of 4 accumulators
            ex_t = {}
            yn_t = {}
            deferred = []  # (due_tick, fn)

            def emit_scores(n):
                kind, G, h, j, gi = cols[n]
                krot = qk_sb[:, NH + h, :]
                qrot = qk_sb[:, h, :]
                q0 = G * QG
                if kind == "w":
                    k0 = max(0, j - 4 * G)
                    w1 = QG
                else:
                    k0 = gi
                    w1 = (gi + 1) * P
                ps_s = ps_s_pool.tile([P, QG], F32)
                nc.tensor.matmul(
                    ps_s[:, k0 * P : w1],
                    krot[:, j * P : (j + 1) * P],
                    qrot[:, q0 + k0 * P : q0 + w1],
                    start=True,
                    stop=True,
                )
                ex = exp_pool.tile([P, QG], BF16)
                nc.scalar.activation(
                    ex[:, k0 * P : w1],
                    ps_s[:, k0 * P : w1],
                    mybir.ActivationFunctionType.Exp,
                    scale=SCALE,
                )
                if j - 4 * G == k0:
                    # causal: zero ex where tk > tq within the diagonal tile
                    sl = slice(k0 * P, (k0 + 1) * P)
                    # keep where tq - tk >= 0 (tk <= tq), else 0
                    nc.gpsimd.affine_select(
                        out=ex[:, sl],
                        in_=ex[:, sl],
                        compare_op=mybir.AluOpType.is_ge,
                        fill=0.0,
                        base=0,
                        pattern=[[1, P]],
                        channel_multiplier=-1,
                    )
                ex_t[n] = ex

            def emit_norm(G, h, gi):
                rec = r_pool.tile([P, 1], F32)
                nc.vector.reciprocal(rec[:], ps_y_t[(G, h)][gi][:, HD : HD + 1])
                y_n = y_pool.tile([P, P], BF16)
                nc.vector.tensor_scalar_mul(
                    y_n[:], ps_y_t[(G, h)][gi][:, 0:HD], rec[:]
                )
                yn_t[(G, h, gi)] = y_n

            def emit_tcopy(G, h, gi):
                y_n = yn_t.pop((G, h, gi))
                ps_t = ps_y_pool.tile(
                    [P, 1024], BF16, name=f"pst_{h}_{G}_{gi}", tag="psy"
                )
                nc.tensor.transpose(ps_t[:, 0:P], y_n[:], ident[:])
                nc.vector.tensor_copy(
                    yt_sb[h][G][:, gi * P : (gi + 1) * P], ps_t[:, 0:P]
                )

            def emit_av(n):
                kind, G, h, j, ngi = cols[n]
                ex = ex_t.pop(n)
                if j == 0 and kind == "w":
                    ps_y_t[(G, h)] = [
                        ps_y_pool.tile(
                            [P, 512], F32, name=f"psy_{h}_{G}_{gi}", tag="psy"
                        )
                        for gi in range(4)
                    ]
                ps_y = ps_y_t[(G, h)]
                voff0 = j * NH + h
                gis = range(4) if kind == "w" else (ngi,)
                done = None
                for gi in gis:
                    i = 4 * G + gi
                    if j <= i:
                        nc.tensor.matmul(
                            ps_y[gi][:, 0:VBW],
                            ex[:, gi * P : (gi + 1) * P],
                            v_sb[:, voff0 * VBW : voff0 * VBW + VBW],
                            start=(j == 0),
                            stop=(j == i),
                        )
                        if j == i:
                            done = gi
                if done is not None:
                    # gi's accumulation just stopped: normalize now,
                    # transpose 2 ticks later (hides the DVE chain)
                    emit_norm(G, h, done)
                    deferred.append((n + 4, "tcopy", (G, h, done)))

            def run_deferred(tick):
                while deferred and deferred[0][0] <= tick:
                    _, kind, args = deferred.pop(0)
                    if kind == "tcopy":
                        G, h, gi = args
                        emit_tcopy(G, h, gi)
                        if G == nqg - 1 and h == NH - 1:
                            # last head of last group: its projections can
                            # only run now; spread og pairs over 2 ticks
                            emit_proj(G, 4 * G + gi, 0)
                            emit_proj(G, 4 * G + gi, 1)
                            deferred.append(
                                (tick + 1, "proj2", (G, 4 * G + gi))
                            )
                            deferred.sort(key=lambda e: e[0])
                    elif kind == "proj2":
                        Gp, tt = args
                        emit_proj(Gp, tt, 2)
                        emit_proj(Gp, tt, 3)

            for n in range(N):
                kind, G, h, j, gi = cols[n]
                emit_scores(n)
                if n < 16:
                    emit_v_chunk(n)
                if n >= 3:
                    emit_av(n - 3)
                run_deferred(n)
                # previous q-group projection weave, every other tick
                if kind == "w" and G > 0 and j in (0, 2, 4, 6):
                    emit_proj(G - 1, 4 * (G - 1) + h, j // 2)
            emit_av(N - 3)
            run_deferred(N)
            emit_av(N - 2)
            emit_av(N - 1)
            tick = N
            while deferred:
                tick += 1
                run_deferred(tick)

    nc.compile()
    return nc


def _rope_tables(t=T):
    inv_freq = 1.0 / (10000.0 ** (np.arange(0, HD, 2, dtype=np.float64) / HD))
    ts = np.arange(t, dtype=np.float64)
    freqs = np.outer(ts, inv_freq)  # [t, 64]
    emb = np.concatenate([freqs, freqs], axis=-1)  # [t, 128]
    cos = np.cos(emb).astype(np.float32)
    sin = np.sin(emb).astype(np.float32)
    cosT = np.ascontiguousarray(cos.T)  # [128, t]
    # rotate_half sign folded into the sin table: rows 0:64 negated
    sinT = np.ascontiguousarray(sin.T)
    sinT[0:64, :] *= -1.0
    return cosT, sinT


def _consts(t=T):
    return _rope_tables(t)


def _split8(a):
    hi = a.astype(ml_dtypes.float8_e4m3)
    lo = (a - hi.astype(np.float32)).astype(ml_dtypes.float8_e4m3)
    return hi, lo


def _core_in_map(x_b, w_qkv, w_out, g, t=T):
    cosT, sinT2 = _consts(t)
    d2 = w_qkv.shape[1]
    q_rows = w_qkv[512 * g : 512 * (g + 1)]
    k_rows = w_qkv[d2 + 512 * g : d2 + 512 * (g + 1)]
    v_rows = w_qkv[2 * d2 + 512 * g : 2 * d2 + 512 * (g + 1)]
    xth, xtl = _split8(np.ascontiguousarray(x_b.T))
    wqh, wql = _split8(
        np.ascontiguousarray(np.concatenate([q_rows, k_rows], axis=0).T)
        * WSCALE
    )
    wvh, wvl = _split8(np.ascontiguousarray(v_rows.T) * WSCALE)
    return {
        "xth": xth,
        "xtl": xtl,
        "wqh": wqh,
        "wql": wql,
        "wvh": wvh,
        "wvl": wvl,
        "wot": np.ascontiguousarray(w_out[:, 512 * g : 512 * (g + 1)].T).astype(
            ml_dtypes.bfloat16
        ),
        "cost": cosT,
        "sint": sinT2,
    }


def kernel(x, w_qkv, w_out):
    global LAST_RESULTS
    x = np.ascontiguousarray(np.asarray(x, dtype=np.float32))
    w_qkv = np.ascontiguousarray(np.asarray(w_qkv, dtype=np.float32))
    w_out = np.ascontiguousarray(np.asarray(w_out, dtype=np.float32))

    if "nc" not in _CACHE:
        _CACHE["nc"] = _build_program()
    nc = _CACHE["nc"]

    B = x.shape[0]
    in_maps = [_core_in_map(x[c // 4], w_qkv, w_out, c % 4) for c in range(8)]
    res = bass_utils.run_bass_kernel_spmd(nc, in_maps, core_ids=list(range(8)))
    LAST_RESULTS = res
    out = np.zeros((B, T, DIM), dtype=np.float32)
    for c in range(8):
        out[c // 4] += res.results[c]["out"]
    return out


if __name__ == "__main__":
    t0 = time.time()
    _CACHE["nc"] = _build_program()
    print(f"program built+compiled in {time.time()-t0:.1f}s")



# revision 3
# speedup vs baseline: 1.0653x; 1.0653x over previous
"""Trainium2 Bass kernel for causal self-attention with RoPE.

Problem: x[2,2048,2048] f32, w_qkv[6144,2048], w_out[2048,2048].
  qkv = x @ w_qkv.T ; split into 16 heads of 128; RoPE on q,k;
  causal softmax attention; out = attn_out @ w_out.T.

Sharding (8 cores): core c -> batch b = c//4, head-group g = c%4
(4 heads each). Each core computes a partial output projection for its
heads; the host sums the 4 partials per batch.

Per-core schedule (engineered against the TimelineSim cost model):

Phase A (fused projections, one streamed pass over x^T in bf16):
  per 256-token group: 8 q/k psum tiles (fp32 PSUM) with RoPE fused at
  eviction -- rotate_half is a 64-partition window swap on DVE (sign
  folded into the host sin table), rotated q/k stored bf16 -- plus 2 v
  psum tiles reusing the same x^T SBUF tiles (v stored bf16 in natural
  [t, hd] layout with a ones column at index 128 for the softmax
  denominator). The v groups and x^T prefetches are software-pipelined
  two token-groups behind the q/k stream (the last two v groups weave
  into phase C's warmup ticks); weight/x loads are ordered so the cold
  start is transfer-bound only.

Phase C (attention + output projection, one flat pipelined stream):
  for each (q-group G, head h): scoresT[tk,tq] = krot.T @ qrot in bf16
  (diagonal tiles width-trimmed), Exp on ACT -> bf16 with causal zeroing
  of the diagonal tile via Pool affine_select, then AV matmuls (129 wide,
  bf16) accumulate [tq, 0:129] in PSUM; col 128 is the denominator.
  Emission is tick-pipelined: AV lags scores by 3 ticks (hides the exp
  chain), per-column-tile eviction (reciprocal + scale on DVE, bf16 PE
  transpose) lags its AV stop by 4 ticks, and the previous q-group's
  output projection (bf16 weights, fp32 PSUM) weaves into every other
  tick. The stream's last head switches to narrow per-gi columns for its
  diagonal region so its evictions + projections interleave instead of
  serializing in the drain.

fp32r/bf16 both run 1 PE cycle/row (moving operand >= 256 wide for
fp32r; any width for bf16). All matmuls accumulate in fp32 PSUM; the
attention core rounds q,k,p,v,y to bf16 (measured rel l2 vs fp64
reference: 5.9e-3).
"""

import os
import sys
import time
from contextlib import ExitStack

import ml_dtypes
import numpy as np

if "/opt/trn_rl_repo" not in sys.path:
    sys.path.insert(0, "/opt/trn_rl_repo")

import concourse.bass as bass  # noqa: E402
import concourse.mybir as mybir  # noqa: E402
import concourse.tile as tile  # noqa: E402
from concourse import bacc  # noqa: E402
from concourse import bass_utils  # noqa: E402
from concourse.masks import make_identity  # noqa: E402

P = 128
T = 2048
DIM = 2048
HD = 128
NH = 4  # heads per core
TGQ = 256  # t-group width for phase A (= fp32r full-rate minimum)
QG = 512  # tq group width in attention
WSCALE = 32.0  # host-side fp8 scale on w_qkv rows
SCALE = float(HD) ** -0.5 / (WSCALE * WSCALE)
F32 = mybir.dt.float32
F32R = mybir.dt.float32r
BF16 = mybir.dt.bfloat16
FP8 = mybir.dt.float8e4
DR = mybir.MatmulPerfMode.DoubleRow
NDP = 8  # DoubleRow d-pairs
VBW = HD + 1  # v block width per (t-tile, head): 128 data + 1 ones col

_CACHE: dict = {}
LAST_RESULTS = None


def _build_program(t=T, dim=DIM):
    ndt = dim // P  # contraction tiles over D (16)
    ntt = t // P  # token tiles (16)
    ntg = t // TGQ  # 8
    nqg = t // QG  # 4
    nog = dim // 512  # 4

    nc = bacc.Bacc("TRN2", target_bir_lowering=False, debug=False)

    xth_d = nc.dram_tensor("xth", [P, dim * t // P], FP8, kind="ExternalInput")
    xtl_d = nc.dram_tensor("xtl", [P, dim * t // P], FP8, kind="ExternalInput")
    wqh_d = nc.dram_tensor("wqh", [P, dim * 2 * NH * HD // P], FP8, kind="ExternalInput")
    wql_d = nc.dram_tensor("wql", [P, dim * 2 * NH * HD // P], FP8, kind="ExternalInput")
    wvh_d = nc.dram_tensor("wvh", [P, dim * NH * HD // P], FP8, kind="ExternalInput")
    wvl_d = nc.dram_tensor("wvl", [P, dim * NH * HD // P], FP8, kind="ExternalInput")
    wo_d = nc.dram_tensor("wot", [NH * HD, dim], BF16, kind="ExternalInput")
    cos_d = nc.dram_tensor("cost", [P, t], F32, kind="ExternalInput")
    sin_d = nc.dram_tensor("sint", [P, t], F32, kind="ExternalInput")
    out_d = nc.dram_tensor("out", [t, dim], F32, kind="ExternalOutput")

    with ExitStack() as ctx:
        tc = ctx.enter_context(tile.TileContext(nc))

        const = ctx.enter_context(tc.tile_pool(name="const", bufs=1))
        ident = const.tile([P, P], BF16)

        # persistent rotated q/k (bf16) and v (bf16, natural layout)
        qk_pool = ctx.enter_context(tc.tile_pool(name="qk", bufs=1))
        qk_sb = qk_pool.tile([P, 2 * NH, t], BF16)
        v_pool = ctx.enter_context(tc.tile_pool(name="vb", bufs=1))
        v_sb = v_pool.tile([P, ntt * NH * VBW], BF16)

        # xta/wv outlive phase A: the last two v groups weave into the
        # attention stream's warmup ticks
        wv_pool = ctx.enter_context(tc.tile_pool(name="wv", bufs=1))
        xt_pool = ctx.enter_context(tc.tile_pool(name="xta", bufs=3))

        # ------- Phase A: fused q/k projection + RoPE + v projection -------
        with (
            tc.tile_pool(name="wqk", bufs=1) as wqk_pool,
            tc.tile_pool(name="trig", bufs=3) as trig_pool,
            tc.tile_pool(name="ropes", bufs=2) as rope_pool,
            tc.tile_pool(name="psqk", bufs=6, space="PSUM") as ps_qk_pool,
            tc.tile_pool(name="psv", bufs=2, space="PSUM") as ps_v_pool,
        ):
            wqh_sb = wqk_pool.tile([P, 4, ndt, 2 * NH * HD // 4], FP8)
            wql_sb = wqk_pool.tile([P, 4, ndt, 2 * NH * HD // 4], FP8)
            wvh_sb = wv_pool.tile([P, ndt, NH * HD], FP8)
            wvl_sb = wv_pool.tile([P, ndt, NH * HD], FP8)
            xth_sb = [
                xt_pool.tile([P, ndt * TGQ], FP8, name=f"xth_{k}", tag="xth")
                for k in range(ntg)
            ]
            xtl_sb = [
                xt_pool.tile([P, ndt * TGQ], FP8, name=f"xtl_{k}", tag="xtl")
                for k in range(ntg)
            ]

            GSZ = ndt * TGQ  # contiguous bytes per partition per group

            def load_xt(k, halves=1, engines=None):
                step = GSZ // halves
                for sb, dram in ((xth_sb, xth_d), (xtl_sb, xtl_d)):
                    for hh in range(halves):
                        o0 = hh * step
                        eng = nc.gpsimd if engines is None else engines[hh]
                        eng.dma_start(
                            sb[k][:, o0 : o0 + step],
                            dram.ap()[:, k * GSZ + o0 : k * GSZ + o0 + step],
                        )

            def load_trig(k, eng=None):
                eng = eng or nc.gpsimd
                t0 = k * TGQ
                cos_t = trig_pool.tile([P, TGQ], F32, name=f"cos_{k}", tag="cos")
                sin_t = trig_pool.tile([P, TGQ], F32, name=f"sin_{k}", tag="sin")
                eng.dma_start(cos_t[:], cos_d.ap()[:, t0 : t0 + TGQ])
                eng.dma_start(sin_t[:], sin_d.ap()[:, t0 : t0 + TGQ])
                return cos_t, sin_t

            # constants first: every later v_sb/ident user depends on
            # them (tile-granular), keep them off the DMA-clogged queue
            nc.gpsimd.memset(
                v_sb[:].rearrange("p (a c) -> p a c", c=VBW)[:, :, HD : HD + 1],
                WSCALE,
            )
            make_identity(nc, ident[:])

            # cold-start DMA order: wqk col-wave 0, xt(tg0), waves 1-3,
            # xt(tg1), wv, xt(tg2); the rest stream during the tg loop.
            NW = 4
            wcol = 2 * NH * HD // NW
            WVSZ = ndt * wcol  # contiguous bytes per partition per wave
            for w in range(NW):
                o0 = w * WVSZ
                nc.sync.dma_start(
                    wqh_sb[:].rearrange("p a b c -> p (a b c)")[
                        :, o0 : o0 + WVSZ
                    ],
                    wqh_d.ap()[:, o0 : o0 + WVSZ],
                )
                if w == 0:
                    load_xt(0, halves=2, engines=[nc.scalar, nc.sync])
                nc.scalar.dma_start(
                    wql_sb[:].rearrange("p a b c -> p (a b c)")[
                        :, o0 : o0 + WVSZ
                    ],
                    wql_d.ap()[:, o0 : o0 + WVSZ],
                )
                if w == 0:
                    load_xt(1, halves=2, engines=[nc.sync, nc.scalar])
                    trig0 = load_trig(0)
                    trig1 = load_trig(1)
            load_xt(2)
            nc.gpsimd.dma_start(
                wvh_sb[:].rearrange("p a b -> p (a b)"), wvh_d.ap()
            )
            nc.gpsimd.dma_start(
                wvl_sb[:].rearrange("p a b -> p (a b)"), wvl_d.ap()
            )
            trigs = {0: trig0, 1: trig1}

            pend_rope = []  # (jt, raw, ps2 deferral) emitted with 1-group lag

            def emit_qk_group(k, jt, cos_t, sin_t):
                t0 = k * TGQ
                xh3 = xth_sb[k][:].rearrange("p (d c) -> p d c", c=TGQ)
                xl3 = xtl_sb[k][:].rearrange("p (d c) -> p d c", c=TGQ)
                wv_, blk = jt // 2, (jt % 2) * P
                cols = slice(blk, blk + P)
                ps = ps_qk_pool.tile([P, TGQ], F32)
                terms = [(wqh_sb, xh3), (wql_sb, xh3), (wqh_sb, xl3)]
                for ti, (wsb, xsb) in enumerate(terms):
                    for dp in range(NDP):
                        nc.tensor.matmul(
                            ps[:],
                            wsb[:, wv_, 2 * dp : 2 * dp + 2, cols],
                            xsb[:, 2 * dp : 2 * dp + 2, :],
                            start=(ti == 0 and dp == 0),
                            stop=(ti == 2 and dp == NDP - 1),
                            perf_mode=DR,
                        )
                raw = rope_pool.tile([P, TGQ], F32R, tag="raw")
                nc.scalar.copy(raw[:], ps[:])
                pend_rope.append((k, jt, raw, cos_t, sin_t))

            def finish_rope():
                # rotate_half as a 64-partition window swap (sign folded
                # into the host sin table), replacing the PE S-matmul
                k, jt, raw, cos_t, sin_t = pend_rope.pop(0)
                t0 = k * TGQ
                rf = raw[:].bitcast(F32)
                sh = rope_pool.tile([P, TGQ], F32, tag="sh")
                nc.vector.tensor_copy(sh[0 : P // 2, :], rf[P // 2 : P, :])
                nc.vector.tensor_copy(sh[P // 2 : P, :], rf[0 : P // 2, :])
                t1 = rope_pool.tile([P, TGQ], F32, tag="t1")
                nc.vector.tensor_mul(t1[:], sh[:], sin_t[:])
                t2 = rope_pool.tile([P, TGQ], F32, tag="t2")
                nc.gpsimd.tensor_mul(t2[:], raw[:].bitcast(F32), cos_t[:])
                nc.vector.tensor_add(qk_sb[:, jt, t0 : t0 + TGQ], t2[:], t1[:])

            def emit_v_group(k, sb):
                tt = k * 2 + sb
                xh3 = xth_sb[k][:].rearrange("p (d c) -> p d c", c=TGQ)
                xl3 = xtl_sb[k][:].rearrange("p (d c) -> p d c", c=TGQ)
                cols = slice(sb * P, (sb + 1) * P)
                ps = ps_v_pool.tile([P, NH * HD], F32, name=f"psv_{tt}", tag="psv")
                terms = [(xh3, wvh_sb), (xh3, wvl_sb), (xl3, wvh_sb)]
                for ti, (xsb, wsb) in enumerate(terms):
                    for dp in range(NDP):
                        nc.tensor.matmul(
                            ps[:],
                            xsb[:, 2 * dp : 2 * dp + 2, cols],
                            wsb[:, 2 * dp : 2 * dp + 2, :],
                            start=(ti == 0 and dp == 0),
                            stop=(ti == 2 and dp == NDP - 1),
                            perf_mode=DR,
                        )
                for h in range(NH):
                    off = (tt * NH + h) * VBW
                    nc.vector.tensor_copy(
                        v_sb[:, off : off + HD], ps[:, h * HD : (h + 1) * HD]
                    )

            # tg0/tg1 interleaved: each wqk column-wave feeds both token
            # groups, so the cold region is PE-bound instead of DMA-bound
            cos0, sin0 = trigs.pop(0)
            cos1, sin1 = trigs.pop(1)
            for jp in range(NH):
                if jp == 1:
                    trigs[2] = load_trig(2)
                if jp == 2:
                    trigs[3] = load_trig(3)
                for k in (0, 1):
                    for jt in (2 * jp, 2 * jp + 1):
                        emit_qk_group(
                            k, jt, cos0 if k == 0 else cos1,
                            sin0 if k == 0 else sin1,
                        )
                        if len(pend_rope) > 1:
                            finish_rope()
            for k in range(2, ntg):
                cos_t, sin_t = trigs.pop(k)
                for jt in range(2 * NH):
                    # v-projection weave: tg k>=2 carries v(k-2) at jts 1,3
                    if jt in (1, 3):
                        emit_v_group(k - 2, (jt - 1) // 2)
                        if jt == 3 and k + 1 < ntg:
                            load_xt(k + 1)
                    if jt == 2 and k + 2 < ntg:
                        trigs[k + 2] = load_trig(k + 2)
                    emit_qk_group(k, jt, cos_t, sin_t)
                    if len(pend_rope) > 1:
                        finish_rope()
            finish_rope()

        # normalized attn out Y^T per (head, q-group), bf16
        yt_pool = ctx.enter_context(tc.tile_pool(name="yt", bufs=1))
        yt_sb = [
            [
                yt_pool.tile([P, QG], BF16, name=f"yt_{h}_{G}", tag=f"yt{h}_{G}")
                for G in range(nqg)
            ]
            for h in range(NH)
        ]

        # ------------- Phase C: attention + output projection -------------
        with (
            tc.tile_pool(name="wo", bufs=1) as wo_pool,
            tc.tile_pool(name="expt", bufs=4) as exp_pool,
            tc.tile_pool(name="ynorm", bufs=6) as y_pool,
            tc.tile_pool(name="recip", bufs=6) as r_pool,
            tc.tile_pool(name="ob", bufs=8) as out_pool,
            tc.tile_pool(name="pss", bufs=2, space="PSUM") as ps_s_pool,
            tc.tile_pool(name="psy", bufs=4, space="PSUM") as ps_y_pool,
            tc.tile_pool(name="pso", bufs=2, space="PSUM") as ps_o_pool,
        ):
            wo_sb = wo_pool.tile([P, nog, NH, 512], BF16)
            for og in range(nog):
                nc.gpsimd.dma_start(
                    wo_sb[:, og, :, :],
                    wo_d.ap()[:, og * 512 : (og + 1) * 512].rearrange(
                        "(h p) c -> p h c", p=P
                    ),
                )

            # last two token-groups' v projection, chunked 4 matmuls per
            # tick across the warmup ticks (pso ring is idle until G1)
            vtail = [(ntg - 2, 0), (ntg - 2, 1), (ntg - 1, 0), (ntg - 1, 1)]
            vtail_ps = {}

            def emit_v_chunk(tick):
                gidx, c = divmod(tick, 4)
                k, sb = vtail[gidx]
                tt = k * 2 + sb
                xh3 = xth_sb[k][:].rearrange("p (d c) -> p d c", c=TGQ)
                xl3 = xtl_sb[k][:].rearrange("p (d c) -> p d c", c=TGQ)
                cols = slice(sb * P, (sb + 1) * P)
                if c == 0:
                    vtail_ps[gidx] = ps_o_pool.tile(
                        [P, NH * HD], F32, name=f"psvt_{tt}", tag="pso"
                    )
                ps = vtail_ps[gidx]
                terms = [(xh3, wvh_sb), (xh3, wvl_sb), (xl3, wvh_sb)]
                units = [(ti, dp) for ti in range(3) for dp in range(NDP)]
                for u in range(6 * c, 6 * c + 6):
                    ti, dp = units[u]
                    xsb, wsb = terms[ti]
                    nc.tensor.matmul(
                        ps[:],
                        xsb[:, 2 * dp : 2 * dp + 2, cols],
                        wsb[:, 2 * dp : 2 * dp + 2, :],
                        start=(u == 0),
                        stop=(u == 23),
                        perf_mode=DR,
                    )
                if c == 3:
                    for h in range(NH):
                        off = (tt * NH + h) * VBW
                        nc.vector.tensor_copy(
                            v_sb[:, off : off + HD], ps[:, h * HD : (h + 1) * HD]
                        )

            def emit_proj(Gp, tt, og):
                ps = ps_o_pool.tile([P, 512], F32, name=f"pso_{og}_{tt}", tag="pso")
                for h in range(NH):
                    nc.tensor.matmul(
                        ps[:],
                        yt_sb[h][Gp][:, (tt % 4) * P : (tt % 4 + 1) * P],
                        wo_sb[:, og, h, :],
                        start=(h == 0),
                        stop=(h == NH - 1),
                    )
                ob = out_pool.tile([P, 512], F32, tag="ob")
                nc.vector.tensor_copy(ob[:], ps[:])
                nc.sync.dma_start(
                    out_d.ap()[tt * P : (tt + 1) * P, og * 512 : (og + 1) * 512],
                    ob[:],
                )

            # --- flat software-pipelined stream over all (G, h, j) ---
            # last head: j>=4G ticks become narrow per-gi columns so its
            # evictions + projections interleave instead of draining late
            cols = []
            for G in range(nqg):
                for h in range(NH):
                    last = G == nqg - 1 and h == NH - 1
                    for j in range(4 * G if last else 4 * G + 4):
                        cols.append(("w", G, h, j, 0))
                    if last:
                        for gi in range(4):
                            for j in range(4 * G, 4 * G + gi + 1):
                                cols.append(("n", G, h, j, gi))
            N = len(cols)
            ps_y_t = {}  # (G,h) -> list of 4 accumulators
            ex_t = {}
            yn_t = {}
            deferred = []  # (due_tick, fn)

            def emit_scores(n):
                kind, G, h, j, gi = cols[n]
                krot = qk_sb[:, NH + h, :]
                qrot = qk_sb[:, h, :]
                q0 = G * QG
                if kind == "w":
                    k0 = max(0, j - 4 * G)
                    w1 = QG
                else:
                    k0 = gi
                    w1 = (gi + 1) * P
                ps_s = ps_s_pool.tile([P, QG], F32)
                nc.tensor.matmul(
                    ps_s[:, k0 * P : w1],
                    krot[:, j * P : (j + 1) * P],
                    qrot[:, q0 + k0 * P : q0 + w1],
                    start=True,
                    stop=True,
                )
                ex = exp_pool.tile([P, QG], BF16)
                nc.scalar.activation(
                    ex[:, k0 * P : w1],
                    ps_s[:, k0 * P : w1],
                    mybir.ActivationFunctionType.Exp,
                    scale=SCALE,
                )
                if j - 4 * G == k0:
                    # causal: zero ex where tk > tq within the diagonal tile
                    sl = slice(k0 * P, (k0 + 1) * P)
                    # keep where tq - tk >= 0 (tk <= tq), else 0
                    nc.gpsimd.affine_select(
                        out=ex[:, sl],
                        in_=ex[:, sl],
                        compare_op=mybir.AluOpType.is_ge,
                        fill=0.0,
                        base=0,
                        pattern=[[1, P]],
                        channel_multiplier=-1,
                    )
                ex_t[n] = ex

            def emit_norm(G, h, gi):
                rec = r_pool.tile([P, 1], F32)
                nc.vector.reciprocal(rec[:], ps_y_t[(G, h)][gi][:, HD : HD + 1])
                y_n = y_pool.tile([P, P], BF16)
                nc.vector.tensor_scalar_mul(
                    y_n[:], ps_y_t[(G, h)][gi][:, 0:HD], rec[:]
                )
                yn_t[(G, h, gi)] = y_n

            def emit_tcopy(G, h, gi):
                y_n = yn_t.pop((G, h, gi))
                ps_t = ps_y_pool.tile(
                    [P, 1024], BF16, name=f"pst_{h}_{G}_{gi}", tag="psy"
                )
                nc.tensor.transpose(ps_t[:, 0:P], y_n[:], ident[:])
                nc.vector.tensor_copy(
                    yt_sb[h][G][:, gi * P : (gi + 1) * P], ps_t[:, 0:P]
                )

            def emit_av(n):
                kind, G, h, j, ngi = cols[n]
                ex = ex_t.pop(n)
                if j == 0 and kind == "w":
                    ps_y_t[(G, h)] = [
                        ps_y_pool.tile(
                            [P, 512], F32, name=f"psy_{h}_{G}_{gi}", tag="psy"
                        )
                        for gi in range(4)
                    ]
                ps_y = ps_y_t[(G, h)]
                voff0 = j * NH + h
                gis = range(4) if kind == "w" else (ngi,)
                done = None
                for gi in gis:
                    i = 4 * G + gi
                    if j <= i:
                        nc.tensor.matmul(
                            ps_y[gi][:, 0:VBW],
                            ex[:, gi * P : (gi + 1) * P],
                            v_sb[:, voff0 * VBW : voff0 * VBW + VBW],
                            start=(j == 0),
                            stop=(j == i),
                        )
                        if j == i:
                            done = gi
                if done is not None:
                    # gi's accumulation just stopped: normalize now,
                    # transpose 2 ticks later (hides the DVE chain)
                    emit_norm(G, h, done)
                    deferred.append((n + 4, "tcopy", (G, h, done)))

            def run_deferred(tick):
                while deferred and deferred[0][0] <= tick:
                    _, kind, args = deferred.pop(0)
                    if kind == "tcopy":
                        G, h, gi = args
                        emit_tcopy(G, h, gi)
                        if G == nqg - 1 and h == NH - 1:
                            # last head of last group: its projections can
                            # only run now; spread og pairs over 2 ticks
                            emit_proj(G, 4 * G + gi, 0)
                            emit_proj(G, 4 * G + gi, 1)
                            deferred.append(
                                (tick + 1, "proj2", (G, 4 * G + gi))
                            )
                            deferred.sort(key=lambda e: e[0])
                    elif kind == "proj2":
                        Gp, tt = args
                        emit_proj(Gp, tt, 2)
                        emit_proj(Gp, tt, 3)

            for n in range(N):
                kind, G, h, j, gi = cols[n]
                emit_scores(n)
                if n < 16:
                    emit_v_chunk(n)
                if n >= 3:
                    emit_av(n - 3)
                run_deferred(n)
                # previous q-group projection weave, every other tick
                if kind == "w" and G > 0 and j in (0, 2, 4, 6):
                    emit_proj(G - 1, 4 * (G - 1) + h, j // 2)
            emit_av(N - 3)
            run_deferred(N)
            emit_av(N - 2)
            emit_av(N - 1)
            tick = N
            while deferred:
                tick += 1
                run_deferred(tick)

    nc.compile()
    return nc


def _rope_tables(t=T):
    inv_freq = 1.0 / (10000.0 ** (np.arange(0, HD, 2, dtype=np.float64) / HD))
    ts = np.arange(t, dtype=np.float64)
    freqs = np.outer(ts, inv_freq)  # [t, 64]
    emb = np.concatenate([freqs, freqs], axis=-1)  # [t, 128]
    cos = np.cos(emb).astype(np.float32)
    sin = np.sin(emb).astype(np.float32)
    cosT = np.ascontiguousarray(cos.T)  # [128, t]
    # rotate_half sign folded into the sin table: rows 0:64 negated
    sinT = np.ascontiguousarray(sin.T)
    sinT[0:64, :] *= -1.0
    return cosT, sinT


def _consts(t=T):
    return _rope_tables(t)


def _split8(a):
    hi = a.astype(ml_dtypes.float8_e4m3)
    lo = (a - hi.astype(np.float32)).astype(ml_dtypes.float8_e4m3)
    return hi, lo


def _part_major_x(xT, ndt=16, P=128, ntg=8, TGQ=256):
    # [d*P+p, k*TGQ+c] -> [p, (k, d, c)] contiguous per group
    a = xT.reshape(ndt, P, ntg, TGQ).transpose(1, 2, 0, 3)
    return np.ascontiguousarray(a.reshape(P, -1))


def _part_major_w(wT, ndt=16, P=128, NW=4):
    # [d*P+p, w*wcol+c] -> [p, (w, d, c)] contiguous per wave
    wcol = wT.shape[1] // NW
    a = wT.reshape(ndt, P, NW, wcol).transpose(1, 2, 0, 3)
    return np.ascontiguousarray(a.reshape(P, -1))


def _part_major_v(wT, ndt=16, P=128):
    a = wT.reshape(ndt, P, -1).transpose(1, 0, 2)
    return np.ascontiguousarray(a.reshape(P, -1))


def _core_in_map(x_b, w_qkv, w_out, g, t=T):
    cosT, sinT2 = _consts(t)
    d2 = w_qkv.shape[1]
    q_rows = w_qkv[512 * g : 512 * (g + 1)]
    k_rows = w_qkv[d2 + 512 * g : d2 + 512 * (g + 1)]
    v_rows = w_qkv[2 * d2 + 512 * g : 2 * d2 + 512 * (g + 1)]
    xth, xtl = _split8(np.ascontiguousarray(x_b.T))
    xth, xtl = _part_major_x(xth), _part_major_x(xtl)
    wqh, wql = _split8(
        np.ascontiguousarray(np.concatenate([q_rows, k_rows], axis=0).T)
        * WSCALE
    )
    wqh, wql = _part_major_w(wqh), _part_major_w(wql)
    wvh, wvl = _split8(np.ascontiguousarray(v_rows.T) * WSCALE)
    wvh, wvl = _part_major_v(wvh), _part_major_v(wvl)
    return {
        "xth": xth,
        "xtl": xtl,
        "wqh": wqh,
        "wql": wql,
        "wvh": wvh,
        "wvl": wvl,
        "wot": np.ascontiguousarray(w_out[:, 512 * g : 512 * (g + 1)].T).astype(
            ml_dtypes.bfloat16
        ),
        "cost": cosT,
        "sint": sinT2,
    }


def kernel(x, w_qkv, w_out):
    global LAST_RESULTS
    x = np.ascontiguousarray(np.asarray(x, dtype=np.float32))
    w_qkv = np.ascontiguousarray(np.asarray(w_qkv, dtype=np.float32))
    w_out = np.ascontiguousarray(np.asarray(w_out, dtype=np.float32))

    if "nc" not in _CACHE:
        _CACHE["nc"] = _build_program()
    nc = _CACHE["nc"]

    B = x.shape[0]
    in_maps = [_core_in_map(x[c // 4], w_qkv, w_out, c % 4) for c in range(8)]
    res = bass_utils.run_bass_kernel_spmd(nc, in_maps, core_ids=list(range(8)))
    LAST_RESULTS = res
    out = np.zeros((B, T, DIM), dtype=np.float32)
    for c in range(8):
        out[c // 4] += res.results[c]["out"]
    return out


if __name__ == "__main__":
    t0 = time.time()
    _CACHE["nc"] = _build_program()
    print(f"program built+compiled in {time.time()-t0:.1f}s")

